# revision 9
# baseline (speedup 1.0000x reference)
"""Trainium2 Bass kernel for nn_MultiHeadAttention_54211077210696.

8-core SPMD sharding: batch (2-way) x heads (4-way).
Core c (b = c//4, j = c%4) computes heads 4j..4j+3 of batch b:
  - Q^T/K^T projections in head-dim-major layout (transpose-free attention)
  - scores computed transposed S'[k,q]; the softmax denominator comes from
    an all-ones block appended to V in the P@V matmul (rows 64..127 = sum)
  - exp runs on ScalarE directly out of PSUM in [128,1024] tiles
  - per-head normalization via reciprocal_approx_fast + cross-partition mul
  - local-head output projection -> fp16 ReduceScatter (per 512-row
    q-chunk, overlapped with compute) -> residual + LayerNorm on the
    scattered rows.
Inputs are pre-cast to fp16 on the host; matmuls run fp16 with fp32 PSUM
accumulation.
"""

import numpy as np
from contextlib import ExitStack

import concourse.bass as bass  # noqa: F401  (registers bass types)
import concourse.tile as tile
from concourse import bacc, mybir
from concourse.bass_utils import run_bass_kernel_spmd

F32 = mybir.dt.float32
F16 = mybir.dt.float16
AF = mybir.ActivationFunctionType
ALU = mybir.AluOpType

N_CORES = 8
GROUPS = [[0, 1, 2, 3], [4, 5, 6, 7]]
B, S, DM = 2, 2048, 1024
HL = 4          # heads per core
DT = 2          # head pairs per core (d-tiles of Q^T/K^T)
KT16 = 16       # 128-row k tiles
EPS = 1e-5

_NC = None


def _emit(nc, tc, ctx, t, dbg=None):
    sing = ctx.enter_context(tc.tile_pool(name="sing", bufs=1))
    xp = ctx.enter_context(tc.tile_pool(name="xp", bufs=4))
    wp = ctx.enter_context(tc.tile_pool(name="wp", bufs=4))
    eop = ctx.enter_context(tc.tile_pool(name="eop", bufs=4))
    rp = ctx.enter_context(tc.tile_pool(name="rp", bufs=4))
    ctp = ctx.enter_context(tc.tile_pool(name="ctp", bufs=4))
    osp = ctx.enter_context(tc.tile_pool(name="osp", bufs=4))
    lnp = ctx.enter_context(tc.tile_pool(name="lnp", bufs=2))
    stp = ctx.enter_context(tc.tile_pool(name="stp", bufs=4))
    psp = ctx.enter_context(tc.tile_pool(name="psp", bufs=4, space="PSUM"))

    QTt = [sing.tile([128, S], F16, tag=f"qt{d}", name=f"qt{d}") for d in range(DT)]
    KTt = [sing.tile([128, S], F16, tag=f"kt{d}", name=f"kt{d}") for d in range(DT)]
    vaug = sing.tile([128, HL * KT16 * 128], F16, tag="vaug")
    wo_sb = [sing.tile([128, DM], F16, tag=f"wo{p}", name=f"wo{p}") for p in range(DT)]
    resid_sb = [sing.tile([128, DM], F32, tag=f"res{qc}", name=f"res{qc}")
                for qc in range(4)]
    eps_t = sing.tile([128, 1], F32, tag="eps")

    nc.vector.memset(eps_t[:], EPS)
    # ones everywhere; V values overwrite columns 0..63 of each 128-block
    nc.vector.memset(vaug[:], 1.0)
    for p in range(DT):
        nc.sync.dma_start(wo_sb[p][:], t["wo"][p * 128:(p + 1) * 128, :])
    for qc in range(4):
        nc.sync.dma_start(resid_sb[qc][:], t["resid"][qc])

    # ---- QKV projections (fp16 in, fp32 PSUM) ----
    # PSUM is managed as 4 slots of [128,1024] (2 banks each); every
    # accumulation chain owns a full bank (sub-bank chain pairs corrupt
    # each other).
    def qk_sweep(x_dram, w_dram, dst):
        pst = [psp.tile([128, 1024], F32, tag="ps2", name="psqk") for _ in range(4)]
        for dmc in range(8):
            xc = xp.tile([128, S], F16, tag="xc")
            nc.sync.dma_start(xc[:], x_dram[dmc * 128:(dmc + 1) * 128, :])
            wc = wp.tile([128, 256], F16, tag="wc")
            nc.sync.dma_start(wc[:], w_dram[dmc * 128:(dmc + 1) * 128, :])
            for d in range(DT):
                for sc in range(4):
                    i = d * 4 + sc
                    nc.tensor.matmul(
                        pst[i // 2][:, (i % 2) * 512:(i % 2) * 512 + 512],
                        wc[:, d * 128:(d + 1) * 128],
                        xc[:, sc * 512:(sc + 1) * 512],
                        start=(dmc == 0), stop=(dmc == 7),
                    )
        for d in range(DT):
            for sc in range(4):
                i = d * 4 + sc
                nc.vector.tensor_copy(dst[d][:, sc * 512:(sc + 1) * 512],
                                      pst[i // 2][:, (i % 2) * 512:(i % 2) * 512 + 512])

    qk_sweep(t["xqT"], t["wq"], QTt)
    qk_sweep(t["xkT"], t["wk"], KTt)

    # V sweep: natural [s, 4*64] layout. One accumulation chain per PSUM
    # bank, so the 16 s-tiles go in two waves of 8.
    for wave in range(2):
        pst = [psp.tile([128, 1024], F32, tag="ps2", name="psv") for _ in range(4)]
        for dmc in range(8):
            xc = xp.tile([128, S], F16, tag="xc")
            nc.sync.dma_start(xc[:], t["xvT"][dmc * 128:(dmc + 1) * 128, :])
            wc = wp.tile([128, 256], F16, tag="wc")
            nc.sync.dma_start(wc[:], t["wv"][dmc * 128:(dmc + 1) * 128, :])
            for i in range(8):
                st = wave * 8 + i
                nc.tensor.matmul(
                    pst[i // 2][:, (i % 2) * 512:(i % 2) * 512 + 256],
                    xc[:, st * 128:(st + 1) * 128],
                    wc[:],
                    start=(dmc == 0), stop=(dmc == 7),
                )
        for i in range(8):
            st = wave * 8 + i
            for h in range(HL):
                nc.vector.tensor_copy(
                    vaug[:, (h * KT16 + st) * 128:(h * KT16 + st) * 128 + 64],
                    pst[i // 2][:, (i % 2) * 512 + h * 64:(i % 2) * 512 + h * 64 + 64],
                )

    if dbg is not None:
        nc.sync.dma_start(dbg["d_qt"], QTt[0][:])
        nc.sync.dma_start(dbg["d_kt"], KTt[0][:])
        nc.sync.dma_start(dbg["d_vaug"], vaug[:])

    # ---- attention + output projection, per 1024-wide q-chunk ----
    for q2 in range(2):
        q0 = q2 * 1024
        ct_pair = []
        for p in range(DT):
            caug = [psp.tile([128, 1024], F32, tag="ps2", name="caug")
                    for _ in range(2)]
            for kt in range(KT16):
                for h in range(2):
                    lo = h * 64
                    pss = psp.tile([128, 1024], F32, tag="ps2", name="pss")
                    for qh in range(2):
                        nc.tensor.matmul(
                            pss[:, qh * 512:(qh + 1) * 512],
                            KTt[p][lo:lo + 64, kt * 128:(kt + 1) * 128],
                            QTt[p][lo:lo + 64, q0 + qh * 512:q0 + (qh + 1) * 512],
                            tile_position=(lo, 0),
                        )
                    eo = eop.tile([128, 1024], F16, tag="eo", name="eo")
                    nc.scalar.activation(eo[:], pss[:], AF.Exp, scale=0.125)
                    if dbg is not None and q2 == 0 and p == 0 and kt == 0 and h == 0:
                        nc.sync.dma_start(dbg["d_eo"], eo[:])
                        dss = eop.tile([128, 1024], F32, tag="dss", name="dss")
                        nc.vector.tensor_copy(dss[:], pss[:])
                        nc.sync.dma_start(dbg["d_ss"], dss[:])
                    hh = p * 2 + h
                    blk = (hh * KT16 + kt) * 128
                    for qh in range(2):
                        nc.tensor.matmul(
                            caug[h][:, qh * 512:(qh + 1) * 512],
                            vaug[:, blk:blk + 128],
                            eo[:, qh * 512:(qh + 1) * 512],
                            start=(kt == 0), stop=(kt == 15),
                        )
            ct = ctp.tile([128, 1024], F16, tag="ct")
            for h in range(2):
                rt = rp.tile([128, 1024], F32, tag="rt")
                nc.vector.tensor_copy(rt[0:64, :], caug[h][64:128, :])
                rt2 = rp.tile([128, 1024], F32, tag="rt2", name="rt2")
                nc.vector.reciprocal_approx_fast(rt2[0:64, :], rt[0:64, :])
                nc.vector.tensor_mul(ct[h * 64:(h + 1) * 64, :],
                                     caug[h][0:64, :], rt2[0:64, :])
            if dbg is not None and q2 == 0 and p == 0:
                dca = ctp.tile([128, 1024], F32, tag="dca", name="dca")
                nc.vector.tensor_copy(dca[:], caug[0][:])
                nc.sync.dma_start(dbg["d_caug"], dca[:])
                nc.sync.dma_start(dbg["d_ct"], ct[:])
            ct_pair.append(ct)

        # local-head output projection; fire RS per 512-row block
        for qt in range(8):
            qc = q2 * 2 + qt // 4
            po = psp.tile([128, 1024], F32, tag="ps2", name="po")
            for dmc in range(2):
                for p in range(DT):
                    nc.tensor.matmul(
                        po[:, dmc * 512:(dmc + 1) * 512],
                        ct_pair[p][:, qt * 128:(qt + 1) * 128],
                        wo_sb[p][:, dmc * 512:(dmc + 1) * 512],
                        start=(p == 0), stop=(p == DT - 1),
                    )
            ost = osp.tile([128, 1024], F16, tag="os")
            nc.vector.tensor_copy(ost[:], po[:])
            nc.sync.dma_start(
                t["rs_in"][qc][(qt % 4) * 128:(qt % 4) * 128 + 128, :],
                ost[:])
            if qt % 4 == 3:
                nc.gpsimd.collective_compute(
                    "ReduceScatter", ALU.add, replica_groups=GROUPS,
                    ins=[t["rs_in"][qc].ap().opt()],
                    outs=[t["rs_out"][qc].ap().opt()])

    # ---- residual + LayerNorm on scattered rows ----
    for qc in range(4):
        ro16 = lnp.tile([128, DM], F16, tag="ro16")
        nc.sync.dma_start(ro16[:], t["rs_out"][qc].ap())
        orow = lnp.tile([128, DM], F32, tag="orow")
        of = lnp.tile([128, DM], F32, tag="of")
        nc.vector.tensor_copy(of[:], ro16[:])
        nc.vector.tensor_add(orow[:], of[:], resid_sb[qc][:])
        stats = stp.tile([128, 2, 6], F32, tag="st")
        for i in range(2):
            nc.vector.bn_stats(stats[:, i, :], orow[:, i * 512:(i + 1) * 512])
        mv = stp.tile([128, 2], F32, tag="mv")
        nc.vector.bn_aggr(mv[:], stats[:])
        rstd = stp.tile([128, 1], F32, tag="rstd")
        nc.scalar.activation(rstd[:], mv[:, 1:2], AF.Sqrt, bias=eps_t[:], scale=1.0)
        nc.vector.reciprocal(rstd[:], rstd[:])
        normed = lnp.tile([128, DM], F32, tag="norm")
        nc.vector.tensor_scalar(normed[:], orow[:], mv[:, 0:1], rstd[:],
                                ALU.subtract, ALU.mult)
        nc.sync.dma_start(t["out"][qc], normed[:])


def _build(with_dbg=False):
    nc = bacc.Bacc("TRN2", target_bir_lowering=False, debug=False,
                   num_devices=N_CORES)
    t = {}
    for name in ("xqT", "xkT", "xvT"):
        t[name] = nc.dram_tensor(name, [DM, S], F16, kind="ExternalInput").ap()
    for name in ("wq", "wk", "wv"):
        t[name] = nc.dram_tensor(name, [DM, 256], F16, kind="ExternalInput").ap()
    t["wo"] = nc.dram_tensor("wo", [256, DM], F16, kind="ExternalInput").ap()
    t["resid"] = nc.dram_tensor("resid", [4, 128, DM], F32, kind="ExternalInput").ap()
    t["out"] = nc.dram_tensor("out", [4, 128, DM], F32, kind="ExternalOutput").ap()
    t["rs_in"] = [nc.dram_tensor(f"rs_in{qc}", [512, DM], F16) for qc in range(4)]
    t["rs_out"] = [nc.dram_tensor(f"rs_out{qc}", [128, DM], F16) for qc in range(4)]

    dbg = None
    if with_dbg:
        dbg = {}
        for nm, shp, dt in [("d_qt", [128, S], F16), ("d_kt", [128, S], F16),
                            ("d_vaug", [128, HL * KT16 * 128], F16),
                            ("d_eo", [128, 1024], F16), ("d_ss", [128, 1024], F32),
                            ("d_caug", [128, 1024], F32), ("d_ct", [128, 1024], F16)]:
            dbg[nm] = nc.dram_tensor(nm, shp, dt, kind="ExternalOutput").ap()

    with tile.TileContext(nc) as tc:
        with ExitStack() as ctx:
            _emit(nc, tc, ctx, t, dbg)
    nc.compile()
    return nc


def kernel(input_Q, input_K, input_V, W_Q, W_K, W_V, W_O):
    global _NC
    if _NC is None:
        _NC = _build()
    nc = _NC

    input_Q = np.asarray(input_Q, dtype=np.float32)
    input_K = np.asarray(input_K, dtype=np.float32)
    input_V = np.asarray(input_V, dtype=np.float32)
    W_Q = np.asarray(W_Q, dtype=np.float32)
    W_K = np.asarray(W_K, dtype=np.float32)
    W_V = np.asarray(W_V, dtype=np.float32)
    W_O = np.asarray(W_O, dtype=np.float32)

    xT = {}
    for nm, x in (("q", input_Q), ("k", input_K), ("v", input_V)):
        for b in range(B):
            xT[nm, b] = np.ascontiguousarray(x[b].T).astype(np.float16)
    in_maps = []
    for c in range(N_CORES):
        b, j = c // 4, c % 4
        resid = np.empty((4, 128, DM), dtype=np.float32)
        for qc in range(4):
            r0 = qc * 512 + j * 128
            resid[qc] = input_Q[b, r0:r0 + 128, :]
        in_maps.append({
            "xqT": xT["q", b], "xkT": xT["k", b], "xvT": xT["v", b],
            "wq": np.ascontiguousarray(W_Q[:, 256 * j:256 * j + 256]).astype(np.float16),
            "wk": np.ascontiguousarray(W_K[:, 256 * j:256 * j + 256]).astype(np.float16),
            "wv": np.ascontiguousarray(W_V[:, 256 * j:256 * j + 256]).astype(np.float16),
            "wo": np.ascontiguousarray(W_O[256 * j:256 * j + 256, :]).astype(np.float16),
            "resid": resid,
        })

    global _last_in_maps
    _last_in_maps = in_maps
    res = run_bass_kernel_spmd(nc, in_maps, core_ids=list(range(N_CORES)))

    out = np.empty((B, S, DM), dtype=np.float32)
    for c in range(N_CORES):
        b, j = c // 4, c % 4
        o = res.results[c]["out"]
        for qc in range(4):
            r0 = qc * 512 + j * 128
            out[b, r0:r0 + 128, :] = o[qc]
    return out


# revision 10
# speedup vs baseline: 1.0431x; 1.0431x over previous
"""Trainium2 Bass kernel for nn_MultiHeadAttention_54211077210696.

8-core SPMD sharding: batch (2-way) x heads (4-way).
Core c (b = c//4, j = c%4) computes heads 4j..4j+3 of batch b:
  - Q^T/K^T projections in head-dim-major layout (transpose-free attention)
  - scores computed transposed S'[k,q]; the softmax denominator comes from
    an all-ones block appended to V in the P@V matmul (rows 64..127 = sum)
  - exp runs on ScalarE directly out of PSUM in [128,1024] tiles
  - per-head normalization via reciprocal_approx_fast + cross-partition mul
  - local-head output projection -> fp16 ReduceScatter (per 512-row
    q-chunk, overlapped with compute) -> residual + LayerNorm on the
    scattered rows.
Inputs are pre-cast to fp16 on the host; matmuls run fp16 with fp32 PSUM
accumulation.
"""

import numpy as np
from contextlib import ExitStack

import concourse.bass as bass  # noqa: F401  (registers bass types)
import concourse.tile as tile
from concourse import bacc, mybir
from concourse.bass_utils import run_bass_kernel_spmd

F32 = mybir.dt.float32
F16 = mybir.dt.float16
AF = mybir.ActivationFunctionType
ALU = mybir.AluOpType

N_CORES = 8
GROUPS = [[0, 1, 2, 3], [4, 5, 6, 7]]
B, S, DM = 2, 2048, 1024
HL = 4          # heads per core
DT = 2          # head pairs per core (d-tiles of Q^T/K^T)
KT16 = 16       # 128-row k tiles
EPS = 1e-5

_NC = None


def _emit(nc, tc, ctx, t, dbg=None):
    sing = ctx.enter_context(tc.tile_pool(name="sing", bufs=1))
    xp = ctx.enter_context(tc.tile_pool(name="xp", bufs=4))
    wp = ctx.enter_context(tc.tile_pool(name="wp", bufs=4))
    eop = ctx.enter_context(tc.tile_pool(name="eop", bufs=4))
    rp = ctx.enter_context(tc.tile_pool(name="rp", bufs=4))
    ctp = ctx.enter_context(tc.tile_pool(name="ctp", bufs=4))
    osp = ctx.enter_context(tc.tile_pool(name="osp", bufs=4))
    lnp = ctx.enter_context(tc.tile_pool(name="lnp", bufs=2))
    stp = ctx.enter_context(tc.tile_pool(name="stp", bufs=4))
    psp = ctx.enter_context(tc.tile_pool(name="psp", bufs=4, space="PSUM"))

    QTt = [sing.tile([128, S], F16, tag=f"qt{d}", name=f"qt{d}") for d in range(DT)]
    KTt = [sing.tile([128, S], F16, tag=f"kt{d}", name=f"kt{d}") for d in range(DT)]
    vaug = sing.tile([128, HL * KT16 * 128], F16, tag="vaug")
    wo_sb = [sing.tile([128, DM], F16, tag=f"wo{p}", name=f"wo{p}") for p in range(DT)]
    resid_sb = [sing.tile([128, DM], F32, tag=f"res{qc}", name=f"res{qc}")
                for qc in range(4)]
    eps_t = sing.tile([128, 1], F32, tag="eps")

    nc.vector.memset(eps_t[:], EPS)
    # ones everywhere; V values overwrite columns 0..63 of each 128-block
    nc.vector.memset(vaug[:], 1.0)
    for p in range(DT):
        nc.gpsimd.dma_start(wo_sb[p][:], t["wo"][p * 128:(p + 1) * 128, :])
    for qc in range(4):
        nc.gpsimd.dma_start(resid_sb[qc][:], t["resid"][qc])

    # ---- QKV projections (fp16 in, fp32 PSUM) ----
    # PSUM is managed as 4 slots of [128,1024] (2 banks each); every
    # accumulation chain owns a full bank (sub-bank chain pairs corrupt
    # each other).
    def qk_sweep(x_dram, w_dram, dst):
        pst = [psp.tile([128, 1024], F32, tag="ps2", name="psqk") for _ in range(4)]
        for dmc in range(8):
            xc = xp.tile([128, S], F16, tag="xc")
            nc.sync.dma_start(xc[:, 0:1024], x_dram[dmc * 128:(dmc + 1) * 128, 0:1024])
            nc.scalar.dma_start(xc[:, 1024:2048], x_dram[dmc * 128:(dmc + 1) * 128, 1024:2048])
            wc = wp.tile([128, 256], F16, tag="wc")
            nc.sync.dma_start(wc[:], w_dram[dmc * 128:(dmc + 1) * 128, :])
            for d in range(DT):
                for sc in range(4):
                    i = d * 4 + sc
                    nc.tensor.matmul(
                        pst[i // 2][:, (i % 2) * 512:(i % 2) * 512 + 512],
                        wc[:, d * 128:(d + 1) * 128],
                        xc[:, sc * 512:(sc + 1) * 512],
                        start=(dmc == 0), stop=(dmc == 7),
                    )
        for d in range(DT):
            for sc in range(4):
                i = d * 4 + sc
                nc.vector.tensor_copy(dst[d][:, sc * 512:(sc + 1) * 512],
                                      pst[i // 2][:, (i % 2) * 512:(i % 2) * 512 + 512])

    qk_sweep(t["xqT"], t["wq"], QTt)
    qk_sweep(t["xkT"], t["wk"], KTt)

    # V sweep: natural [s, 4*64] layout. One accumulation chain per PSUM
    # bank, so the 16 s-tiles go in two waves of 8.
    for wave in range(2):
        pst = [psp.tile([128, 1024], F32, tag="ps2", name="psv") for _ in range(4)]
        for dmc in range(8):
            xc = xp.tile([128, S], F16, tag="xc")
            nc.sync.dma_start(xc[:, 0:1024], t["xvT"][dmc * 128:(dmc + 1) * 128, 0:1024])
            nc.scalar.dma_start(xc[:, 1024:2048], t["xvT"][dmc * 128:(dmc + 1) * 128, 1024:2048])
            wc = wp.tile([128, 256], F16, tag="wc")
            nc.sync.dma_start(wc[:], t["wv"][dmc * 128:(dmc + 1) * 128, :])
            for i in range(8):
                st = wave * 8 + i
                nc.tensor.matmul(
                    pst[i // 2][:, (i % 2) * 512:(i % 2) * 512 + 256],
                    xc[:, st * 128:(st + 1) * 128],
                    wc[:],
                    start=(dmc == 0), stop=(dmc == 7),
                )
        for i in range(8):
            st = wave * 8 + i
            for h in range(HL):
                nc.vector.tensor_copy(
                    vaug[:, (h * KT16 + st) * 128:(h * KT16 + st) * 128 + 64],
                    pst[i // 2][:, (i % 2) * 512 + h * 64:(i % 2) * 512 + h * 64 + 64],
                )

    if dbg is not None:
        nc.sync.dma_start(dbg["d_qt"], QTt[0][:])
        nc.sync.dma_start(dbg["d_kt"], KTt[0][:])
        nc.sync.dma_start(dbg["d_vaug"], vaug[:])

    # ---- attention + output projection, per 1024-wide q-chunk ----
    for q2 in range(2):
        q0 = q2 * 1024
        ct_pair = []
        for p in range(DT):
            caug = [psp.tile([128, 1024], F32, tag="ps2", name="caug")
                    for _ in range(2)]

            def accum(kt, h, eo):
                hh = p * 2 + h
                blk = (hh * KT16 + kt) * 128
                for qh in range(2):
                    nc.tensor.matmul(
                        caug[h][:, qh * 512:(qh + 1) * 512],
                        vaug[:, blk:blk + 128],
                        eo[:, qh * 512:(qh + 1) * 512],
                        start=(kt == 0), stop=(kt == 15),
                    )

            pend = None  # (kt, h, eo) whose accum is deferred one step
            for kt in range(KT16):
                for h in range(2):
                    lo = h * 64
                    pss = psp.tile([128, 1024], F32, tag="ps2", name="pss")
                    for qh in range(2):
                        nc.tensor.matmul(
                            pss[:, qh * 512:(qh + 1) * 512],
                            KTt[p][lo:lo + 64, kt * 128:(kt + 1) * 128],
                            QTt[p][lo:lo + 64, q0 + qh * 512:q0 + (qh + 1) * 512],
                            tile_position=(lo, 0),
                        )
                    eo = eop.tile([128, 1024], F16, tag="eo", name="eo")
                    nc.scalar.activation(eo[:], pss[:], AF.Exp, scale=0.125)
                    if dbg is not None and q2 == 0 and p == 0 and kt == 0 and h == 0:
                        nc.sync.dma_start(dbg["d_eo"], eo[:])
                        dss = eop.tile([128, 1024], F32, tag="dss", name="dss")
                        nc.vector.tensor_copy(dss[:], pss[:])
                        nc.sync.dma_start(dbg["d_ss"], dss[:])
                    if pend is not None:
                        accum(*pend)
                    pend = (kt, h, eo)
            accum(*pend)
            ct = ctp.tile([128, 1024], F16, tag="ct")
            for h in range(2):
                rt = rp.tile([128, 1024], F32, tag="rt")
                nc.vector.tensor_copy(rt[0:64, :], caug[h][64:128, :])
                rt2 = rp.tile([128, 1024], F32, tag="rt2", name="rt2")
                nc.vector.reciprocal_approx_fast(rt2[0:64, :], rt[0:64, :])
                nc.vector.tensor_mul(ct[h * 64:(h + 1) * 64, :],
                                     caug[h][0:64, :], rt2[0:64, :])
            if dbg is not None and q2 == 0 and p == 0:
                dca = ctp.tile([128, 1024], F32, tag="dca", name="dca")
                nc.vector.tensor_copy(dca[:], caug[0][:])
                nc.sync.dma_start(dbg["d_caug"], dca[:])
                nc.sync.dma_start(dbg["d_ct"], ct[:])
            ct_pair.append(ct)

        # local-head output projection; fire RS per 512-row block
        for qt in range(8):
            qc = q2 * 2 + qt // 4
            po = psp.tile([128, 1024], F32, tag="ps2", name="po")
            for dmc in range(2):
                for p in range(DT):
                    nc.tensor.matmul(
                        po[:, dmc * 512:(dmc + 1) * 512],
                        ct_pair[p][:, qt * 128:(qt + 1) * 128],
                        wo_sb[p][:, dmc * 512:(dmc + 1) * 512],
                        start=(p == 0), stop=(p == DT - 1),
                    )
            ost = osp.tile([128, 1024], F16, tag="os")
            nc.vector.tensor_copy(ost[:], po[:])
            nc.sync.dma_start(
                t["rs_in"][qc][(qt % 4) * 128:(qt % 4) * 128 + 128, :],
                ost[:])
            if qt % 4 == 3:
                nc.gpsimd.collective_compute(
                    "ReduceScatter", ALU.add, replica_groups=GROUPS,
                    ins=[t["rs_in"][qc].ap().opt()],
                    outs=[t["rs_out"][qc].ap().opt()])

    # ---- residual + LayerNorm on scattered rows ----
    for qc in range(4):
        ro16 = lnp.tile([128, DM], F16, tag="ro16")
        nc.gpsimd.dma_start(ro16[:], t["rs_out"][qc].ap())
        orow = lnp.tile([128, DM], F32, tag="orow")
        of = lnp.tile([128, DM], F32, tag="of")
        nc.vector.tensor_copy(of[:], ro16[:])
        nc.vector.tensor_add(orow[:], of[:], resid_sb[qc][:])
        stats = stp.tile([128, 2, 6], F32, tag="st")
        for i in range(2):
            nc.vector.bn_stats(stats[:, i, :], orow[:, i * 512:(i + 1) * 512])
        mv = stp.tile([128, 2], F32, tag="mv")
        nc.vector.bn_aggr(mv[:], stats[:])
        rstd = stp.tile([128, 1], F32, tag="rstd")
        nc.scalar.activation(rstd[:], mv[:, 1:2], AF.Sqrt, bias=eps_t[:], scale=1.0)
        nc.vector.reciprocal(rstd[:], rstd[:])
        normed = lnp.tile([128, DM], F32, tag="norm")
        nc.vector.tensor_scalar(normed[:], orow[:], mv[:, 0:1], rstd[:],
                                ALU.subtract, ALU.mult)
        nc.sync.dma_start(t["out"][qc], normed[:])


def _build(with_dbg=False):
    nc = bacc.Bacc("TRN2", target_bir_lowering=False, debug=False,
                   num_devices=N_CORES)
    t = {}
    for name in ("xqT", "xkT", "xvT"):
        t[name] = nc.dram_tensor(name, [DM, S], F16, kind="ExternalInput").ap()
    for name in ("wq", "wk", "wv"):
        t[name] = nc.dram_tensor(name, [DM, 256], F16, kind="ExternalInput").ap()
    t["wo"] = nc.dram_tensor("wo", [256, DM], F16, kind="ExternalInput").ap()
    t["resid"] = nc.dram_tensor("resid", [4, 128, DM], F32, kind="ExternalInput").ap()
    t["out"] = nc.dram_tensor("out", [4, 128, DM], F32, kind="ExternalOutput").ap()
    t["rs_in"] = [nc.dram_tensor(f"rs_in{qc}", [512, DM], F16) for qc in range(4)]
    t["rs_out"] = [nc.dram_tensor(f"rs_out{qc}", [128, DM], F16) for qc in range(4)]

    dbg = None
    if with_dbg:
        dbg = {}
        for nm, shp, dt in [("d_qt", [128, S], F16), ("d_kt", [128, S], F16),
                            ("d_vaug", [128, HL * KT16 * 128], F16),
                            ("d_eo", [128, 1024], F16), ("d_ss", [128, 1024], F32),
                            ("d_caug", [128, 1024], F32), ("d_ct", [128, 1024], F16)]:
            dbg[nm] = nc.dram_tensor(nm, shp, dt, kind="ExternalOutput").ap()

    with tile.TileContext(nc) as tc:
        with ExitStack() as ctx:
            _emit(nc, tc, ctx, t, dbg)
    nc.compile()
    return nc


def kernel(input_Q, input_K, input_V, W_Q, W_K, W_V, W_O):
    global _NC
    if _NC is None:
        _NC = _build()
    nc = _NC

    input_Q = np.asarray(input_Q, dtype=np.float32)
    input_K = np.asarray(input_K, dtype=np.float32)
    input_V = np.asarray(input_V, dtype=np.float32)
    W_Q = np.asarray(W_Q, dtype=np.float32)
    W_K = np.asarray(W_K, dtype=np.float32)
    W_V = np.asarray(W_V, dtype=np.float32)
    W_O = np.asarray(W_O, dtype=np.float32)

    xT = {}
    for nm, x in (("q", input_Q), ("k", input_K), ("v", input_V)):
        for b in range(B):
            xT[nm, b] = np.ascontiguousarray(x[b].T).astype(np.float16)
    in_maps = []
    for c in range(N_CORES):
        b, j = c // 4, c % 4
        resid = np.empty((4, 128, DM), dtype=np.float32)
        for qc in range(4):
            r0 = qc * 512 + j * 128
            resid[qc] = input_Q[b, r0:r0 + 128, :]
        in_maps.append({
            "xqT": xT["q", b], "xkT": xT["k", b], "xvT": xT["v", b],
            "wq": np.ascontiguousarray(W_Q[:, 256 * j:256 * j + 256]).astype(np.float16),
            "wk": np.ascontiguousarray(W_K[:, 256 * j:256 * j + 256]).astype(np.float16),
            "wv": np.ascontiguousarray(W_V[:, 256 * j:256 * j + 256]).astype(np.float16),
            "wo": np.ascontiguousarray(W_O[256 * j:256 * j + 256, :]).astype(np.float16),
            "resid": resid,
        })

    global _last_in_maps
    _last_in_maps = in_maps
    res = run_bass_kernel_spmd(nc, in_maps, core_ids=list(range(N_CORES)))

    out = np.empty((B, S, DM), dtype=np.float32)
    for c in range(N_CORES):
        b, j = c // 4, c % 4
        o = res.results[c]["out"]
        for qc in range(4):
            r0 = qc * 512 + j * 128
            out[b, r0:r0 + 128, :] = o[qc]
    return out
